# revision 1
# baseline (speedup 1.0000x reference)
"""Trainium2 Bass kernel for nn_CrossAttentionLayer (sigmoid cross-attention).

Sharding: pure data-parallel over the batch dim — core c computes batch c
(bs=8 across 8 NeuronCores, zero collectives).

Per-core device program (batch-local shapes: text (512,1024), av (1024,1024)):
  1. LayerNorm both streams in token-major layout (bn_stats/bn_aggr on DVE,
     rsqrt via ACT-Sqrt + DVE reciprocal, fused (x-mu)*rinv on DVE), bf16 out.
  2. Round-trip the normalized activations through DRAM and reload transposed
     via the DMA xbar (feature-major tiles for the matmul contraction dim).
  3. v projection first (token-major v, K=1 ones-matmul for its bias), then a
     fused loop over head pairs: q/k projection for the pair's feature block
     immediately followed by that pair's attention, so PE projection work
     overlaps ACT sigmoid work.
  4. Attention: S^T = kT_h^T @ qT_h (K=64, the two heads of a pair on disjoint
     PE row groups), kv-blocks paired into 2-bank PSUM tiles so each ACT
     sigmoid covers 1024 elements; out^T accumulated over kv with v stationary
     (pair on disjoint col groups via tile_position); attention-mean over
     heads via a pairwise bf16 add tree split across DVE and GpSimd.
  5. Outputs transposed back to token-major with PE transposes (spread through
     the loop for out, at the end for attn-mean) and SWDGE cast-stores.
"""
import numpy as np
import ml_dtypes

import concourse.bacc as bacc
import concourse.mybir as mybir
import concourse.tile as tile
from concourse.bass_utils import run_bass_kernel_spmd

bf16 = ml_dtypes.bfloat16
BF = mybir.dt.bfloat16
F32 = mybir.dt.float32
AF = mybir.ActivationFunctionType
ALU = mybir.AluOpType

NW = 512      # num_word (queries)
NV = 1024     # num_valid (keys/values)
D = 1024      # d_model
H = 16        # heads
DK = 64       # head dim
NCORES = 8

_CACHE: dict = {}


def _build_program():
    nc = bacc.Bacc("TRN2", target_bir_lowering=False, debug=False)

    xt_d = nc.declare_dram_parameter("xt", [NW, D], BF, isOutput=False)
    xa_d = nc.declare_dram_parameter("xa", [NV, D], BF, isOutput=False)
    wq_d = nc.declare_dram_parameter("wqT", [D, D], BF, isOutput=False)
    wk_d = nc.declare_dram_parameter("wkT", [D, D], BF, isOutput=False)
    wv_d = nc.declare_dram_parameter("wvT", [D, D], BF, isOutput=False)
    bq_d = nc.declare_dram_parameter("bq", [D], F32, isOutput=False)
    bk_d = nc.declare_dram_parameter("bk", [D], F32, isOutput=False)
    bv_d = nc.declare_dram_parameter("bv", [1, D], BF, isOutput=False)
    id_d = nc.declare_dram_parameter("ident", [128, 128], BF, isOutput=False)

    out_d = nc.declare_dram_parameter("out", [NW, D], F32, isOutput=True)
    am_d = nc.declare_dram_parameter("am", [NW, NV], F32, isOutput=True)

    that_dram = nc.dram_tensor("that_scratch", [NW, D], BF)
    ahat_dram = nc.dram_tensor("ahat_scratch", [NV, D], BF)

    with tile.TileContext(nc) as tc:
        import contextlib
        with contextlib.ExitStack() as ctx:
            const_p = ctx.enter_context(tc.tile_pool(name="const", bufs=1))
            in_p = ctx.enter_context(tc.tile_pool(name="in", bufs=3))
            stat_p = ctx.enter_context(tc.tile_pool(name="stat", bufs=24))
            hat_p = ctx.enter_context(tc.tile_pool(name="hat", bufs=3))
            tT_p = ctx.enter_context(tc.tile_pool(name="tT", bufs=8))
            aT_p = ctx.enter_context(tc.tile_pool(name="aT", bufs=8))
            w_p = ctx.enter_context(tc.tile_pool(name="w", bufs=22))
            wv_p = ctx.enter_context(tc.tile_pool(name="wv", bufs=10))
            qT_p = ctx.enter_context(tc.tile_pool(name="qT", bufs=3))
            kT_p = ctx.enter_context(tc.tile_pool(name="kT", bufs=3))
            v_p = ctx.enter_context(tc.tile_pool(name="v", bufs=8))
            pt_p = ctx.enter_context(tc.tile_pool(name="pt", bufs=10))
            mean_p = ctx.enter_context(tc.tile_pool(name="mean", bufs=22))
            otb_p = ctx.enter_context(tc.tile_pool(name="otb", bufs=3))
            row_p = ctx.enter_context(tc.tile_pool(name="row", bufs=8))
            amf_p = ctx.enter_context(tc.tile_pool(name="amf", bufs=4))

            eps_t = const_p.tile([128, 1], F32)
            nc.gpsimd.memset(eps_t[:], 1e-5)
            zero_t = const_p.tile([128, 1], F32)
            nc.gpsimd.memset(zero_t[:], 0.0)
            ones_t = const_p.tile([1, 128], BF)
            nc.gpsimd.memset(ones_t[:], 1.0)
            sixt_t = const_p.tile([128, 1], F32)
            nc.gpsimd.memset(sixt_t[:], 1.0 / H)
            ident = const_p.tile([128, 128], BF)
            nc.sync.dma_start(ident[:], id_d[:])

            # per-partition bias slices: [128, 8] with [p, fb] = b[fb*128 + p]
            bq_sb = const_p.tile([128, 8], F32)
            nc.sync.dma_start(bq_sb[:], bq_d[:].rearrange("(a p) -> p a", p=128))
            bk_sb = const_p.tile([128, 8], F32)
            nc.sync.dma_start(bk_sb[:], bk_d[:].rearrange("(a p) -> p a", p=128))
            bv_sb = const_p.tile([1, D], BF)
            nc.sync.dma_start(bv_sb[:], bv_d[:])

            # ---------------- Phase 1: LayerNorm (token-major) ----------------
            def layer_norm(src_dram, dst_dram, ntiles):
                for i in range(ntiles):
                    tx = in_p.tile([128, D], BF, tag="ln_in")
                    nc.sync.dma_start(tx[:], src_dram[i * 128:(i + 1) * 128, :])
                    st = stat_p.tile([128, 12], F32, tag="st12")
                    nc.vector.bn_stats(st[:, 0:6], tx[:, 0:512])
                    nc.vector.bn_stats(st[:, 6:12], tx[:, 512:1024])
                    mv = stat_p.tile([128, 2], F32, tag="mv")
                    nc.vector.bn_aggr(mv[:], st[:])
                    std = stat_p.tile([128, 1], F32, tag="std")
                    nc.scalar.activation(std[:], mv[:, 1:2], AF.Sqrt, bias=eps_t[:])
                    rinv = stat_p.tile([128, 1], F32, tag="rinv")
                    nc.vector.reciprocal(rinv[:], std[:])
                    th = hat_p.tile([128, D], BF, tag="hat")
                    nc.vector.tensor_scalar(
                        th[:], tx[:], mv[:, 0:1], rinv[:], ALU.subtract, ALU.mult
                    )
                    nc.sync.dma_start(dst_dram[i * 128:(i + 1) * 128, :], th[:])

            layer_norm(xa_d, ahat_dram, NV // 128)
            layer_norm(xt_d, that_dram, NW // 128)

            # ------------- Phase 2: reload transposed via DMA xbar -------------
            aT = []
            for db in range(8):
                t = aT_p.tile([128, NV], BF, tag="aT", name=f"aT{db}")
                aT.append(t)
            tT = []
            for db in range(8):
                t = tT_p.tile([128, NW], BF, tag="tT", name=f"tT{db}")
                tT.append(t)
            for db in range(8):
                nc.sync.dma_start(aT[db][:],
                                  ahat_dram[:, db * 128:(db + 1) * 128],
                                  transpose=True)
            for db in range(8):
                nc.sync.dma_start(tT[db][:], that_dram[:, db * 128:(db + 1) * 128],
                                  transpose=True)

            with (
                tc.tile_pool(name="work_ps", bufs=2, space="PSUM") as work_ps,
                tc.tile_pool(name="s_ps", bufs=2, space="PSUM") as s_ps,
                tc.tile_pool(name="o_ps", bufs=2, space="PSUM") as o_ps,
            ):
                # ------------- Phase 3: v projection (token-major v[j, f]) -------------
                v = [v_p.tile([128, D], BF, tag="v", name=f"v{i}") for i in range(8)]
                for fh in range(2):
                    wvs = []
                    for db in range(8):
                        w = wv_p.tile([128, 512], BF, tag="wv")
                        nc.sync.dma_start(
                            w[:], wv_d[db * 128:(db + 1) * 128,
                                       fh * 512:(fh + 1) * 512])
                        wvs.append(w)
                    for jb in range(8):
                        ps = work_ps.tile([128, 512], F32, tag="work")
                        for db in range(8):
                            nc.tensor.matmul(
                                ps[:], aT[db][:, jb * 128:(jb + 1) * 128],
                                wvs[db][:], start=(db == 0), stop=False)
                        nc.tensor.matmul(
                            ps[:], ones_t[:], bv_sb[0:1, fh * 512:(fh + 1) * 512],
                            start=False, stop=True)
                        nc.vector.tensor_copy(v[jb][:, fh * 512:(fh + 1) * 512], ps[:])

                # persistent token-major output accumulators
                out_row = [row_p.tile([128, D], BF, tag="row", name=f"orow{i}")
                           for i in range(4)]
                am_row = [row_p.tile([128, NV], BF, tag="row", name=f"arow{i}")
                          for i in range(4)]

                lv = [[None] * 5 for _ in range(4)]   # mean tree per kv pair
                alt = 0

                def mean_insert(jp, c):
                    nonlocal alt
                    k = 0
                    while lv[jp][k] is not None:
                        prev = lv[jp][k]
                        lv[jp][k] = None
                        nt = mean_p.tile([128, 1024], BF, tag="mean")
                        eng = nc.vector if alt % 2 == 0 else nc.gpsimd
                        alt += 1
                        eng.tensor_add(nt[:], prev[:], c[:])
                        c = nt
                        k += 1
                    lv[jp][k] = c

                # ---------- fused per-head-pair loop: projections + attention ----------
                for fb in range(8):
                    # q/k weight strips for this feature block: [d, fb*128 ±]
                    wqs, wks = [], []
                    for db in range(8):
                        wq = w_p.tile([128, 128], BF, tag="w")
                        nc.sync.dma_start(
                            wq[:], wq_d[db * 128:(db + 1) * 128,
                                        fb * 128:(fb + 1) * 128])
                        wqs.append(wq)
                        wk = w_p.tile([128, 128], BF, tag="w")
                        nc.sync.dma_start(
                            wk[:], wk_d[db * 128:(db + 1) * 128,
                                        fb * 128:(fb + 1) * 128])
                        wks.append(wk)

                    ps = work_ps.tile([128, 512], F32, tag="work")
                    for db in range(8):
                        nc.tensor.matmul(ps[:], wqs[db][:], tT[db][:],
                                         start=(db == 0), stop=(db == 7))
                    qt = qT_p.tile([128, NW], BF, tag="qt")
                    nc.vector.tensor_scalar_add(qt[:], ps[:], bq_sb[:, fb:fb + 1])

                    kt = kT_p.tile([128, NV], BF, tag="kt")
                    for jh in range(2):
                        ps = work_ps.tile([128, 512], F32, tag="work")
                        for db in range(8):
                            nc.tensor.matmul(
                                ps[:], wks[db][:],
                                aT[db][:, jh * 512:(jh + 1) * 512],
                                start=(db == 0), stop=(db == 7))
                        nc.vector.tensor_scalar_add(
                            kt[:, jh * 512:(jh + 1) * 512], ps[:],
                            bk_sb[:, fb:fb + 1])

                    # attention for heads (2*fb, 2*fb+1); kv blocks in pairs
                    h0, h1 = 2 * fb, 2 * fb + 1
                    pt0, pt1 = [], []
                    for jp in range(4):
                        je, jo = 2 * jp, 2 * jp + 1
                        for r0, plist in ((0, pt0), (64, pt1)):
                            sp = s_ps.tile([128, 1024], F32, tag="sp")
                            nc.tensor.matmul(
                                sp[:, 0:512],
                                kt[r0:r0 + 64, je * 128:(je + 1) * 128],
                                qt[r0:r0 + 64, :], start=True, stop=True)
                            nc.tensor.matmul(
                                sp[:, 512:1024],
                                kt[r0:r0 + 64, jo * 128:(jo + 1) * 128],
                                qt[r0:r0 + 64, :], start=True, stop=True)
                            pt = pt_p.tile([128, 1024], BF, tag="pt")
                            nc.scalar.activation(pt[:], sp[:], AF.Sigmoid,
                                                 bias=zero_t[:], scale=0.125)
                            plist.append(pt)

                    ops0 = o_ps.tile([128, 512], F32, tag="ops")
                    ops1 = o_ps.tile([128, 512], F32, tag="ops")
                    for jp in range(4):
                        for half in range(2):
                            jb = 2 * jp + half
                            nc.tensor.matmul(
                                ops0[0:64, :], v[jb][:, h0 * 64:(h0 + 1) * 64],
                                pt0[jp][:, half * 512:(half + 1) * 512],
                                start=(jb == 0), stop=(jb == 7),
                                tile_position=(0, 0))
                            nc.tensor.matmul(
                                ops1[64:128, :], v[jb][:, h1 * 64:(h1 + 1) * 64],
                                pt1[jp][:, half * 512:(half + 1) * 512],
                                start=(jb == 0), stop=(jb == 7),
                                tile_position=(0, 64))

                    otb = otb_p.tile([128, 512], BF, tag="otb")
                    nc.vector.tensor_copy(otb[0:64, :], ops0[0:64, :])
                    nc.vector.tensor_copy(otb[64:128, :], ops1[64:128, :])
                    for ib in range(4):
                        tp = work_ps.tile([128, 128], BF, tag="work")
                        nc.tensor.transpose(
                            tp[:], otb[:, ib * 128:(ib + 1) * 128], ident[:])
                        nc.scalar.activation(
                            out_row[ib][:, fb * 128:(fb + 1) * 128], tp[:],
                            AF.Copy)

                    for jp in range(4):
                        mean_insert(jp, pt0[jp])
                        mean_insert(jp, pt1[jp])

                # ---------------- attn-mean finalization ----------------
                for jp in range(4):
                    fin = amf_p.tile([128, 1024], BF, tag="amf")
                    nc.vector.tensor_scalar_mul(fin[:], lv[jp][4][:], sixt_t[:])
                    for half in range(2):
                        jb = 2 * jp + half
                        for ib in range(4):
                            tp = work_ps.tile([128, 128], BF, tag="work")
                            nc.tensor.transpose(
                                tp[:],
                                fin[:, half * 512 + ib * 128:
                                    half * 512 + (ib + 1) * 128],
                                ident[:])
                            nc.scalar.activation(
                                am_row[ib][:, jb * 128:(jb + 1) * 128], tp[:],
                                AF.Copy)

                for ib in range(4):
                    nc.gpsimd.dma_start(out_d[ib * 128:(ib + 1) * 128, :],
                                        out_row[ib][:])
                    nc.gpsimd.dma_start(am_d[ib * 128:(ib + 1) * 128, :],
                                        am_row[ib][:])

    nc.compile()
    return nc


def _get_program():
    if "nc" not in _CACHE:
        _CACHE["nc"] = _build_program()
    return _CACHE["nc"]


def kernel(text, av_feat, tn_w, tn_b, an_w, an_b, Wq, bq, Wk, bk, Wv, bv):
    text = np.asarray(text, dtype=np.float32)
    av_feat = np.asarray(av_feat, dtype=np.float32)
    tn_w = np.asarray(tn_w, dtype=np.float32)
    tn_b = np.asarray(tn_b, dtype=np.float32)
    an_w = np.asarray(an_w, dtype=np.float32)
    an_b = np.asarray(an_b, dtype=np.float32)
    Wq = np.asarray(Wq, dtype=np.float32)
    bq = np.asarray(bq, dtype=np.float32)
    Wk = np.asarray(Wk, dtype=np.float32)
    bk = np.asarray(bk, dtype=np.float32)
    Wv = np.asarray(Wv, dtype=np.float32)
    bv = np.asarray(bv, dtype=np.float32)

    bs = text.shape[0]
    assert bs == NCORES and text.shape == (NCORES, NW, D)
    assert av_feat.shape == (NCORES, NV, D)

    # Fold LN affine into the projection weights (host-side, O(d^2)):
    #   q = ((x_hat*w + b) @ Wq.T + bq) = x_hat @ (Wq*w).T + (bq + Wq @ b)
    wqT = np.ascontiguousarray((Wq * tn_w[None, :]).T).astype(bf16)
    wkT = np.ascontiguousarray((Wk * an_w[None, :]).T).astype(bf16)
    wvT = np.ascontiguousarray((Wv * an_w[None, :]).T).astype(bf16)
    bq_eff = (bq + Wq @ tn_b).astype(np.float32)
    bk_eff = (bk + Wk @ an_b).astype(np.float32)
    bv_eff = (bv + Wv @ an_b).astype(bf16).reshape(1, D)
    ident = np.eye(128).astype(bf16)

    nc = _get_program()

    in_maps = []
    for c in range(NCORES):
        in_maps.append({
            "xt": text[c].astype(bf16),
            "xa": av_feat[c].astype(bf16),
            "wqT": wqT, "wkT": wkT, "wvT": wvT,
            "bq": bq_eff, "bk": bk_eff, "bv": bv_eff,
            "ident": ident,
        })

    res = run_bass_kernel_spmd(nc, in_maps, core_ids=list(range(NCORES)))
    out = np.stack([res.results[c]["out"] for c in range(NCORES)])
    am = np.stack([res.results[c]["am"] for c in range(NCORES)])
    return out, am



# revision 18
# speedup vs baseline: 1.1295x; 1.1295x over previous
"""Trainium2 Bass kernel for nn_CrossAttentionLayer (sigmoid cross-attention).

Sharding: pure data-parallel over the batch dim — core c computes batch c
(bs=8 across 8 NeuronCores, zero collectives).

Per-core device program (batch-local shapes: text (512,1024), av (1024,1024)):
  1. Whole weight matrices loaded with one DMA each into [128, (chunk, col)]
     SBUF tiles (8 feature chunks of 128 folded along the free dim).
  2. LayerNorm token-major (bn_stats/bn_aggr, ACT sqrt, DVE reciprocal,
     fused (x-mu)*rinv tensor_scalar), one DRAM writeback DMA per stream,
     transposed reload via the DMA xbar into feature-major tiles.
  3. q projections for all 8 feature blocks run while the av stream is still
     normalizing; v projection token-major; per-fb loop does k projection,
     per-kv-block scores S^T = k_h^T q_h (two heads per feature block),
     ACT sigmoid, then the out einsum in M=128 orientation:
     out[q,dk] += attn^T[kv,q]^T v[kv,dk] (N=64 moving) — token-major PSUM,
     no output transposes.
  4. attn-mean over heads via per-kv-block pairwise bf16 add tree
     (DVE/GpSimd), transposed to token-major with PE transposes at the end.
  5. Outputs stored with casting SWDGE DMAs (bf16 SBUF -> f32 DRAM).
"""
import numpy as np
import ml_dtypes

import concourse.bacc as bacc
import concourse.mybir as mybir
import concourse.tile as tile
from concourse.bass_utils import run_bass_kernel_spmd

bf16 = ml_dtypes.bfloat16
BF = mybir.dt.bfloat16
F32 = mybir.dt.float32
AF = mybir.ActivationFunctionType
ALU = mybir.AluOpType

NW = 512      # num_word (queries)
NV = 1024     # num_valid (keys/values)
D = 1024      # d_model
H = 16        # heads
DK = 64       # head dim
NCORES = 8

_CACHE: dict = {}


def _build_program(with_v_bias: bool):
    nc = bacc.Bacc("TRN2", target_bir_lowering=False, debug=False)

    xt_d = nc.declare_dram_parameter("xt", [NW, D], BF, isOutput=False)
    xa_d = nc.declare_dram_parameter("xa", [NV, D], BF, isOutput=False)
    wq_d = nc.declare_dram_parameter("wqT", [D, D], BF, isOutput=False)
    wk_d = nc.declare_dram_parameter("wkT", [D, D], BF, isOutput=False)
    wv_d = nc.declare_dram_parameter("wvT", [D, D], BF, isOutput=False)
    bq_d = nc.declare_dram_parameter("bq", [D], F32, isOutput=False)
    bk_d = nc.declare_dram_parameter("bk", [D], F32, isOutput=False)
    bv_d = nc.declare_dram_parameter("bv", [1, D], BF, isOutput=False)
    id_d = nc.declare_dram_parameter("ident", [128, 128], BF, isOutput=False)

    out_d = nc.declare_dram_parameter("out", [NW, D], F32, isOutput=True)
    am_d = nc.declare_dram_parameter("am", [NW, NV], F32, isOutput=True)

    that_dram = nc.dram_tensor("that_scratch", [NW, D], BF)
    ahat_dram = nc.dram_tensor("ahat_scratch", [NV, D], BF)

    with tile.TileContext(nc) as tc:
        import contextlib
        with contextlib.ExitStack() as ctx:
            const_p = ctx.enter_context(tc.tile_pool(name="const", bufs=1))
            w_p = ctx.enter_context(tc.tile_pool(name="w", bufs=1))
            xT_p = ctx.enter_context(tc.tile_pool(name="xT", bufs=1))
            v_p = ctx.enter_context(tc.tile_pool(name="v", bufs=1))
            stat_p = ctx.enter_context(tc.tile_pool(name="stat", bufs=12))

            eps_t = const_p.tile([128, 1], F32)
            nc.gpsimd.memset(eps_t[:], 1e-5)
            ident = const_p.tile([128, 128], BF)
            nc.sync.dma_start(ident[:], id_d[:])
            bq_sb = const_p.tile([128, 8], F32)
            nc.sync.dma_start(bq_sb[:], bq_d[:].rearrange("(a p) -> p a", p=128))
            bk_sb = const_p.tile([128, 8], F32)
            nc.sync.dma_start(bk_sb[:], bk_d[:].rearrange("(a p) -> p a", p=128))
            if with_v_bias:
                bv_sb = const_p.tile([1, D], BF)
                nc.sync.dma_start(bv_sb[:], bv_d[:])
                ones_t = const_p.tile([1, 128], BF)
                nc.gpsimd.memset(ones_t[:], 1.0)

            # whole weight matrices, one DMA each: w_sb[p, db*1024 + c] =
            # W^T[db*128 + p, c]
            wq_sb = w_p.tile([128, 8 * D], BF, name="wq_sb")
            nc.sync.dma_start(wq_sb[:],
                              wq_d[:, :].rearrange("(a p) d -> p a d", p=128))
            wk_sb = w_p.tile([128, 8 * D], BF, name="wk_sb")
            nc.sync.dma_start(wk_sb[:],
                              wk_d[:, :].rearrange("(a p) d -> p a d", p=128))
            wv_sb = w_p.tile([128, 8 * D], BF, name="wv_sb")
            nc.sync.dma_start(wv_sb[:],
                              wv_d[:, :].rearrange("(a p) d -> p a d", p=128))

            # feature-major normalized activations
            tT = xT_p.tile([128, 8 * NW], BF, name="tT")   # [p, fb*512 + t]
            aT = xT_p.tile([128, 8 * NV], BF, name="aT")   # [p, db*1024 + j]

            # ---------------- Phase 1: LayerNorm (token-major) ----------------
            def layer_norm(src_dram, dst_dram, xin, ngroups, xT_tile, ncols):
                nc.sync.dma_start(
                    xin[:], src_dram[:, :].rearrange("(g p) d -> p g d", p=128))
                for g in range(ngroups):
                    sl = slice(g * D, (g + 1) * D)
                    st = stat_p.tile([128, 12], F32, tag="st12")
                    nc.vector.bn_stats(st[:, 0:6], xin[:, g * D:g * D + 512])
                    nc.vector.bn_stats(st[:, 6:12], xin[:, g * D + 512:(g + 1) * D])
                    mv = stat_p.tile([128, 2], F32, tag="mv")
                    nc.vector.bn_aggr(mv[:], st[:])
                    std = stat_p.tile([128, 1], F32, tag="std")
                    nc.scalar.activation(std[:], mv[:, 1:2], AF.Sqrt, bias=eps_t[:])
                    rinv = stat_p.tile([128, 1], F32, tag="rinv")
                    nc.vector.reciprocal(rinv[:], std[:])
                    nc.vector.tensor_scalar(
                        xin[:, sl], xin[:, sl], mv[:, 0:1], rinv[:],
                        ALU.subtract, ALU.mult)
                nc.sync.dma_start(
                    dst_dram[:, :].rearrange("(g p) d -> p g d", p=128), xin[:])
                # transposed reload via DMA xbar
                for db in range(8):
                    nc.sync.dma_start(
                        xT_tile[:, db * ncols:(db + 1) * ncols],
                        dst_dram[:, db * 128:(db + 1) * 128], transpose=True)

            with tc.tile_pool(name="ln", bufs=1) as ln_p:
                xt_sb = ln_p.tile([128, 4 * D], BF, name="xt_sb")
                xa_sb = ln_p.tile([128, 8 * D], BF, name="xa_sb")
                layer_norm(xt_d, that_dram, xt_sb, 4, tT, NW)
                layer_norm(xa_d, ahat_dram, xa_sb, 8, aT, NV)

            # fb-phase pools open after the LN pool closes so they can reuse
            # its SBUF range (execution order is dependency-driven, so q/v
            # projections still overlap the av-stream LayerNorm).
            qt_p = ctx.enter_context(tc.tile_pool(name="qt", bufs=8))
            kt_p = ctx.enter_context(tc.tile_pool(name="kt", bufs=2))
            pt_p = ctx.enter_context(tc.tile_pool(name="pt", bufs=8))
            s1_p = ctx.enter_context(tc.tile_pool(name="s1", bufs=9))
            tree_p = ctx.enter_context(tc.tile_pool(name="tree", bufs=13))
            fin_p = ctx.enter_context(tc.tile_pool(name="fin", bufs=4))
            row_p = ctx.enter_context(tc.tile_pool(name="row", bufs=1))

            with (
                tc.tile_pool(name="pq_ps", bufs=2, space="PSUM") as pq_ps,
                tc.tile_pool(name="sp_ps", bufs=2, space="PSUM") as sp_ps,
                tc.tile_pool(name="o_ps", bufs=2, space="PSUM") as o_ps,
            ):
                # ---- q projections for all feature blocks (overlap LN-a) ----
                qts = []
                for fb in range(8):
                    ps = pq_ps.tile([128, NW], F32, tag="pq")
                    for db in range(8):
                        nc.tensor.matmul(
                            ps[:], wq_sb[:, db * D + fb * 128:db * D + (fb + 1) * 128],
                            tT[:, db * NW:(db + 1) * NW],
                            start=(db == 0), stop=(db == 7))
                    qt = qt_p.tile([128, NW], BF, tag="qt")
                    nc.vector.tensor_scalar_add(qt[:], ps[:], bq_sb[:, fb:fb + 1])
                    qts.append(qt)

                # ---------------- v projection (token-major) ----------------
                v = []
                for jb in range(8):
                    ps = sp_ps.tile([128, D], F32, tag="sp")
                    for fh in range(2):
                        psl = ps[:, fh * 512:(fh + 1) * 512]
                        for db in range(8):
                            nc.tensor.matmul(
                                psl,
                                aT[:, db * NV + jb * 128:db * NV + (jb + 1) * 128],
                                wv_sb[:, db * D + fh * 512:db * D + (fh + 1) * 512],
                                start=(db == 0), stop=(db == 7 and not with_v_bias))
                        if with_v_bias:
                            nc.tensor.matmul(
                                psl, ones_t[:],
                                bv_sb[0:1, fh * 512:(fh + 1) * 512],
                                start=False, stop=True)
                    vt = v_p.tile([128, D], BF, name=f"v{jb}")
                    nc.vector.tensor_copy(vt[:], ps[:])
                    v.append(vt)

                # persistent token-major output accumulator [p, qb*1024 + d]
                out_row = row_p.tile([128, 4 * D], BF, name="out_row")
                am_row = [row_p.tile([128, NV], BF, name=f"am_row{i}")
                          for i in range(4)]

                lv = [[None] * 4 for _ in range(4)]
                alt = [0]

                def tree_add(dst, a, b):
                    eng = nc.gpsimd if alt[0] % 2 == 1 else nc.vector
                    alt[0] += 1
                    eng.tensor_add(dst, a, b)

                def mean_insert(jp, c):
                    k = 0
                    while lv[jp][k] is not None:
                        prev = lv[jp][k]
                        lv[jp][k] = None
                        nt = tree_p.tile([128, NV], BF, tag="tree")
                        tree_add(nt[:], prev[:], c[:])
                        c = nt
                        k += 1
                    lv[jp][k] = c

                # ------------- fused per-head-pair (fb) loop -------------
                for fb in range(8):
                    h0, h1 = 2 * fb, 2 * fb + 1
                    # k projection for this feature block
                    kps = sp_ps.tile([128, NV], F32, tag="sp")
                    for jh in range(2):
                        for db in range(8):
                            nc.tensor.matmul(
                                kps[:, jh * 512:(jh + 1) * 512],
                                wk_sb[:, db * D + fb * 128:db * D + (fb + 1) * 128],
                                aT[:, db * NV + jh * 512:db * NV + (jh + 1) * 512],
                                start=(db == 0), stop=(db == 7))
                    kt = kt_p.tile([128, NV], BF, tag="kt")
                    nc.vector.tensor_scalar_add(kt[:], kps[:], bk_sb[:, fb:fb + 1])

                    qt = qts[fb]
                    o_ps_t = o_ps.tile([128, 512], F32, tag="ops")
                    pts = []
                    s1s = [s1_p.tile([128, NV], BF, tag="s1", name=f"s1_{fb}_{i}")
                           for i in range(4)]
                    for jb in range(8):
                        # scores S^T[kv, q] for both heads of the pair
                        sp = sp_ps.tile([128, NV], F32, tag="sp")
                        nc.tensor.matmul(
                            sp[:, 0:512], kt[0:64, jb * 128:(jb + 1) * 128],
                            qt[0:64, :], start=True, stop=True)
                        nc.tensor.matmul(
                            sp[:, 512:1024], kt[64:128, jb * 128:(jb + 1) * 128],
                            qt[64:128, :], start=True, stop=True)
                        pt = pt_p.tile([128, NV], BF, tag="pt")
                        nc.scalar.activation(pt[:], sp[:], AF.Sigmoid, scale=0.125)
                        pts.append(pt)

                        # out[q, dk] accumulation, token-major (M=128, N=64).
                        # o_ps_t is a single PSUM bank: matmul start=True
                        # zeroes the whole 2KB bank, so exactly one start (the
                        # very first matmul) and one stop (the very last) per
                        # fb iteration.
                        for h in range(2):
                            hh = (h0 if h == 0 else h1)
                            for qb in range(4):
                                nc.tensor.matmul(
                                    o_ps_t[:, qb * 128 + h * 64:
                                           qb * 128 + h * 64 + 64],
                                    pt[:, h * 512 + qb * 128:
                                       h * 512 + (qb + 1) * 128],
                                    v[jb][:, hh * 64:(hh + 1) * 64],
                                    start=(jb == 0 and h == 0 and qb == 0),
                                    stop=(jb == 7 and h == 1 and qb == 3),
                                    skip_group_check=True)

                        # head-pair partial sum for attn-mean
                        jp, half = jb // 2, jb % 2
                        nc.vector.tensor_add(
                            s1s[jp][:, half * 512:(half + 1) * 512],
                            pt[:, 0:512], pt[:, 512:1024])

                    # out_ps -> out_row[:, qb*1024 + fb*128 + h*64 + dk]
                    nc.vector.tensor_copy(
                        out_row[:, :].rearrange(
                            "p (qb d) -> p qb d", qb=4)[:, :, fb * 128:(fb + 1) * 128],
                        o_ps_t[:])

                    for jp in range(4):
                        mean_insert(jp, s1s[jp])

                # ---------------- attn-mean finalization ----------------
                for jp in range(4):
                    fin = fin_p.tile([128, NV], BF, tag="fin", name=f"fin{jp}")
                    nc.vector.tensor_scalar_mul(fin[:], lv[jp][3][:], 1.0 / H)
                    lv[jp][3] = fin

                for qb in range(4):
                    for grp in range(2):
                        tp = (pq_ps if grp == 0 else o_ps).tile(
                            [128, 512], BF, tag="pq" if grp == 0 else "ops")
                        for j in range(4):
                            jb = grp * 4 + j
                            jp, half = jb // 2, jb % 2
                            nc.tensor.transpose(
                                tp[:, j * 128:(j + 1) * 128],
                                lv[jp][3][:, half * 512 + qb * 128:
                                          half * 512 + (qb + 1) * 128],
                                ident[:])
                        nc.vector.tensor_copy(
                            am_row[qb][:, grp * 512:(grp + 1) * 512], tp[:])

                # ---------------- outputs (casting SWDGE stores) ----------------
                nc.gpsimd.dma_start(
                    out_d[:, :].rearrange("(g p) d -> p g d", p=128), out_row[:])
                for qb in range(4):
                    nc.gpsimd.dma_start(am_d[qb * 128:(qb + 1) * 128, :],
                                        am_row[qb][:])

    nc.compile()
    return nc


def _get_program(with_v_bias: bool = False):
    key = ("nc", with_v_bias)
    if key not in _CACHE:
        _CACHE[key] = _build_program(with_v_bias)
    _CACHE["last"] = _CACHE[key]
    return _CACHE[key]


def kernel(text, av_feat, tn_w, tn_b, an_w, an_b, Wq, bq, Wk, bk, Wv, bv):
    text = np.asarray(text, dtype=np.float32)
    av_feat = np.asarray(av_feat, dtype=np.float32)
    tn_w = np.asarray(tn_w, dtype=np.float32)
    tn_b = np.asarray(tn_b, dtype=np.float32)
    an_w = np.asarray(an_w, dtype=np.float32)
    an_b = np.asarray(an_b, dtype=np.float32)
    Wq = np.asarray(Wq, dtype=np.float32)
    bq = np.asarray(bq, dtype=np.float32)
    Wk = np.asarray(Wk, dtype=np.float32)
    bk = np.asarray(bk, dtype=np.float32)
    Wv = np.asarray(Wv, dtype=np.float32)
    bv = np.asarray(bv, dtype=np.float32)

    bs = text.shape[0]
    assert bs == NCORES and text.shape == (NCORES, NW, D)
    assert av_feat.shape == (NCORES, NV, D)

    # Fold LN affine into the projection weights (host-side, O(d^2)):
    #   q = ((x_hat*w + b) @ Wq.T + bq) = x_hat @ (Wq*w).T + (bq + Wq @ b)
    wqT = np.ascontiguousarray((Wq * tn_w[None, :]).T).astype(bf16)
    wkT = np.ascontiguousarray((Wk * an_w[None, :]).T).astype(bf16)
    wvT = np.ascontiguousarray((Wv * an_w[None, :]).T).astype(bf16)
    bq_eff = (bq + Wq @ tn_b).astype(np.float32)
    bk_eff = (bk + Wk @ an_b).astype(np.float32)
    bv_eff = (bv + Wv @ an_b).astype(np.float32)
    with_v_bias = bool(np.any(bv_eff))
    ident = np.eye(128).astype(bf16)

    nc = _get_program(with_v_bias)

    in_maps = []
    for c in range(NCORES):
        in_maps.append({
            "xt": text[c].astype(bf16),
            "xa": av_feat[c].astype(bf16),
            "wqT": wqT, "wkT": wkT, "wvT": wvT,
            "bq": bq_eff, "bk": bk_eff,
            "bv": bv_eff.astype(bf16).reshape(1, D),
            "ident": ident,
        })

    res = run_bass_kernel_spmd(nc, in_maps, core_ids=list(range(NCORES)))
    out = np.stack([res.results[c]["out"] for c in range(NCORES)])
    am = np.stack([res.results[c]["am"] for c in range(NCORES)])
    return out, am
